# revision 53
# baseline (speedup 1.0000x reference)
"""Trainium2 Bass kernel for the Chambolle-Pock-style primal/dual stencil loop.

Math (per image, H=W=1024, EPS=0.5, TAU=0.5, 10 iterations):
    u = sigmoid(o/EPS); q = 0
    repeat 10x:
        q  = relu(q - TAU*(vf1*Dy(u) + vf0*Dx(u)))   # forward diffs, zero pad
        Tq = BDy(vf1*q) + BDx(vf0*q)                  # backward diffs, zero pad
        u  = sigmoid((o - Tq)/EPS)
    return (o - Tq)/EPS

Rescaled recurrence (qh = 2*sqrt(2)*q, g = vf/sqrt(2), gs = g0+g1, o2 = 2*o,
t = tanh(s/2) with s = 2(o - Tq); u zero-padding becomes (-1)-padding of t):
    qh = relu(qh + gs*t - g1*t[y+1] - g0*t[x+1])
    s  = o2 - A + A[y-1] - B + B[x-1]        (A = g1*qh, B = g0*qh, pads 0)
    t  = tanh(s/2)
and the final output is s itself.

Engine split:
  * all tensors live in SBUF as fp16 -> DVE tensor_tensor runs in 2x_1P mode
    (0.52 ns/elem).  DVE does ONLY the 5 products per plane
    (m1 = gs*t, m2 = g1*t[y+1], m3 = g0*t[x+1], A = g1*qh, B = g0*qh).
  * every add/sub is an accumulation on the Tensor engine as +-identity
    matmuls into PSUM (exact fp32, 1 cycle/row for fp16):
      dual:   psum_q = I*qh + I*m1 - I*m2 - I*m3
      primal: psum_s = I*o2 - I*A + I*A_pr - I*B + I*B_pc
    Shifted operands are shifted SBUF access patterns of the rhs; the two
    cross-partition boundary rows (t[y+1] at plane 7, A[y-1] at plane 0) are
    handled by PE partition-shift matrices instead of SBUF->SBUF DMAs:
      plane 7:  -I*m2[7]  ->  NSd*z + E127*g1row,  z = g1s7 * t[plane 0]
                (NSd[k,m] = -[k==m+1]; E127 supplies the t=-1 bottom pad term)
      plane 0:  +I*A_pr[0] -> Spu*A[plane 7]       (Spu[k,m] = +[k==m-1])
  * the "+qh" dual term and the "+B_pc - B" primal pair for some planes are
    pre-folded into single SBUF tensors on the Pool engine / VectorE
    (SB_DVE/SB_POOL below), trading cheap slack on those engines for PE
    matmuls until all four engines are near-equally loaded.
  * ScalarE consumes PSUM directly: qh = relu(psum_q), t = tanh(psum_s/2).
  * last iteration: psum_s goes out via ACT copy to fp16 + DMA (the host
    upcasts; the fp16 output quantization is negligible vs iteration noise).

The per-iteration blocks are emitted in a software-pipelined order (duals
2,3,4,5,1,6,0,7; primals 3,4,5,2,6,1,7,0; A/B products lagged one dual) that
is self-consistent across iterations: the first dual of an iteration
depends on a tanh from the MIDDLE of the previous one, so iterations
overlap by ~9 blocks, no engine queue ever drains at the boundary, and the
Tensor engine keeps its high-frequency pstate (zero steady-state gaps,
PE/DVE both ~91% busy).

Layout: image row y = 8*p + i -> partition p (0..127), plane i (0..7).
Column shifts use guard columns (t col W = -1, B col 0 = 0).

Precision: fp16 products + exact fp32 accumulation measures rel-L2 ~1.0e-2
on hardware against the fp32 reference (gate: 2e-2), matching a
bit-faithful numpy simulation of the op sequence. The dominant term is fp16
quantization noise amplified by the relu's chaotic decision boundaries; no
single tensor dominates.

Sharding: pure data parallel, one image per NeuronCore (B=8 over 8 cores);
g0/g1/gs and the matmul matrices broadcast to all cores.
"""

import numpy as np

import concourse.bacc as bacc
import concourse.mybir as mybir
from concourse.tile import TileContext
from concourse import bass_utils

F16 = mybir.dt.float16
F32 = mybir.dt.float32
AF = mybir.ActivationFunctionType

B, H, W = 8, 1024, 1024
P = 128          # SBUF partitions
NP = H // P      # planes per partition = 8
WG = W + 1       # plane width incl. one guard column
HW = W // 2      # matmul moving-dim max = one PSUM bank of fp32
MAXITER = 10

# planes whose primal "+B_pc - B" pair is pre-folded into one SBUF tensor
# (sB) on VectorE / Pool instead of two PE matmuls, for engine balance.
# Pool gets the planes whose ab->primal schedule gap is large (its ops are
# ~2.1us, so they need lead time to stay off the PE's critical path).
SB_DVE = (3, 4, 5)
SB_POOL = (1, 2)
SB_SLOT = {p: j for j, p in enumerate(SB_DVE + SB_POOL)}

_CACHE = {}
LAST_RESULTS = None  # BassKernelResults of the most recent run (for test.py)


def _build():
    nc = bacc.Bacc("TRN2", target_bir_lowering=False, debug=False)

    o2_d = nc.dram_tensor("o2", [H, W], F16, kind="ExternalInput").ap()
    g0_d = nc.dram_tensor("g0", [H, W], F16, kind="ExternalInput").ap()
    g1_d = nc.dram_tensor("g1", [H, W], F16, kind="ExternalInput").ap()
    gs_d = nc.dram_tensor("gs", [H, W], F16, kind="ExternalInput").ap()
    g1s_d = nc.dram_tensor("g1s", [P, W], F16, kind="ExternalInput").ap()
    # mats: [eye | ney | nsd | spu | e127] stationary matrices
    mats_d = nc.dram_tensor("mats", [P, 5 * P], F16, kind="ExternalInput").ap()
    out_d = nc.dram_tensor("out", [H, W], F16, kind="ExternalOutput").ap()

    # (H, W) -> (p, i, x) with y = 8*p + i
    o2_v = o2_d.rearrange("(p i) x -> p i x", i=NP)
    g0_v = g0_d.rearrange("(p i) x -> p i x", i=NP)
    g1_v = g1_d.rearrange("(p i) x -> p i x", i=NP)
    gs_v = gs_d.rearrange("(p i) x -> p i x", i=NP)
    out_v = out_d.rearrange("(p i) x -> p i x", i=NP)

    v = nc.vector
    act = nc.scalar
    mm = nc.tensor.matmul

    with TileContext(nc) as tc:
        with (
            tc.tile_pool(name="main", bufs=1) as pool,
            tc.tile_pool(name="psq", bufs=2, space="PSUM") as psqp,
            tc.tile_pool(name="pss", bufs=2, space="PSUM") as pssp,
            tc.tile_pool(name="sout", bufs=2) as sop,
        ):
            o2t = pool.tile([P, NP, W], F16)
            g0t = pool.tile([P, NP, W], F16)
            g1t = pool.tile([P, NP, W], F16)
            gst = pool.tile([P, NP, W], F16)
            g1st = pool.tile([P, W], F16)
            qht = pool.tile([P, NP, W], F16)
            m1t = pool.tile([P, NP, W], F16)
            m2t = pool.tile([P, NP, W], F16)
            m3t = pool.tile([P, NP, W], F16)
            # t-state: planes 0..7 = t; col W = -1 guard for x+1 reads
            sut = pool.tile([P, NP, WG], F16)
            # A[0..7]
            abt = pool.tile([P, NP, W], F16)
            # B: col 0 = zero guard for x-1 reads, cols 1..W = B
            bbt = pool.tile([P, NP, WG], F16)
            # sB = B_pc - B for the folded planes (SB_SLOT order)
            sbt = pool.tile([P, len(SB_DVE) + len(SB_POOL), W], F16)
            mats = pool.tile([P, 5 * P], F16)

            eyt = mats[:, 0 * P : 1 * P]   # +identity
            nyt = mats[:, 1 * P : 2 * P]   # -identity
            nsd = mats[:, 2 * P : 3 * P]   # out[m] = -in[m+1]
            spu = mats[:, 3 * P : 4 * P]   # out[m] = +in[m-1]
            e127 = mats[:, 4 * P : 5 * P]  # out[127] = +in[127], else 0

            v.memset(sut[:, :, W : W + 1], -1.0)
            v.memset(bbt[:, :, 0:1], 0.0)
            nc.sync.dma_start(out=mats[:, :], in_=mats_d)
            nc.sync.dma_start(out=g1st[:, :], in_=g1s_d)
            # per-plane loads + per-plane initial tanh, in first-use order,
            # so the pipeline starts ~20us earlier than one bulk load would
            for i in (2, 3, 4, 5, 1, 6, 0, 7):
                nc.sync.dma_start(out=o2t[:, i, :], in_=o2_v[:, i, :])
                nc.sync.dma_start(out=gst[:, i, :], in_=gs_v[:, i, :])
                nc.sync.dma_start(out=g1t[:, i, :], in_=g1_v[:, i, :])
                nc.sync.dma_start(out=g0t[:, i, :], in_=g0_v[:, i, :])
                act.activation(sut[:, i, 0:W], o2t[:, i, :], AF.Tanh, scale=0.5)


            def emit_mprods(i, pair=False):
                # pair=True: plane i and i+1 products in one DVE pass each
                n = 2 if pair else 1
                s = slice(i, i + n)
                v.tensor_mul(m1t[:, s, :], gst[:, s, :], sut[:, s, 0:W])
                if i < NP - 1:
                    s1 = slice(i + 1, i + 1 + n)
                    v.tensor_mul(m2t[:, s, :], g1t[:, s, :], sut[:, s1, 0:W])
                else:
                    # z = g1s7 * t[plane 0];  -I*m2[7] == NSd*z + E127*g1row
                    v.tensor_mul(m2t[:, i, :], g1st[:, :], sut[:, 0, 0:W])
                v.tensor_mul(m3t[:, s, :], g0t[:, s, :], sut[:, s, 1 : W + 1])

            def emit_fold(i, it):
                # Pool folds the dual's +qh term into m1 so PE skips I*qh
                if it > 0:
                    nc.gpsimd.tensor_add(
                        m1t[:, i, :], m1t[:, i, :], qht[:, i, :]
                    )

            def emit_dual_mm(i, it, warm=0):
                psq = psqp.tile([P, W], F32)
                # pstate warmup: discarded matmuls (the real group's
                # start=True resets the bank) keep PE busy through the
                # cold-start window so it reaches 2.4 GHz before real work
                for _ in range(warm):
                    mm(psq[:, 0:HW], eyt, mats[:, 0:HW], start=True, stop=True)
                for h in (0, HW):
                    sl = slice(h, h + HW)
                    mm(psq[:, sl], eyt, m1t[:, i, sl], start=True, stop=False)
                    if i < NP - 1:
                        mm(psq[:, sl], nyt, m2t[:, i, sl], start=False, stop=False)
                    else:
                        mm(psq[:, sl], nsd, m2t[:, i, sl], start=False, stop=False)
                        mm(psq[:, sl], e127, g1t[:, i, sl], start=False, stop=False)
                    mm(psq[:, sl], nyt, m3t[:, i, sl], start=False, stop=True)
                act.activation(qht[:, i, :], psq[:, :], AF.Relu)

            def emit_ab(i):
                v.tensor_mul(abt[:, i, :], g1t[:, i, :], qht[:, i, :])
                v.tensor_mul(bbt[:, i, 1 : W + 1], g0t[:, i, :], qht[:, i, :])

            def emit_sb(i):
                eng = v if i in SB_DVE else nc.gpsimd
                eng.tensor_sub(
                    sbt[:, SB_SLOT[i], :], bbt[:, i, 0:W], bbt[:, i, 1 : W + 1]
                )

            def emit_primal(i, last):
                pss = pssp.tile([P, W], F32)
                for h in (0, HW):
                    sl = slice(h, h + HW)
                    mm(pss[:, sl], eyt, o2t[:, i, sl], start=True, stop=False)
                    mm(pss[:, sl], nyt, abt[:, i, sl], start=False, stop=False)
                    if i > 0:
                        mm(pss[:, sl], eyt, abt[:, i - 1, sl], start=False, stop=False)
                    else:
                        mm(pss[:, sl], spu, abt[:, NP - 1, sl], start=False, stop=False)
                    if i in SB_SLOT:
                        mm(pss[:, sl], eyt, sbt[:, SB_SLOT[i], sl],
                           start=False, stop=True)
                    else:
                        mm(pss[:, sl], nyt, bbt[:, i, slice(h + 1, h + 1 + HW)],
                           start=False, stop=False)
                        mm(pss[:, sl], eyt, bbt[:, i, sl], start=False, stop=True)
                if last:
                    sot = sop.tile([P, W], F16)
                    act.copy(sot[:, :], pss[:, :])
                    nc.sync.dma_start(out=out_v[:, i, :], in_=sot[:, :])
                else:
                    act.activation(sut[:, i, 0:W], pss[:, :], AF.Tanh, scale=0.5)

            # Software-pipelined block order.  d<i> = plane-i dual
            # (m-products, matmuls, relu), ab<j> = plane-j A/B products
            # (lagged one dual so they never stall DVE), p<i> = plane-i
            # primal (matmuls, tanh/copy-out).
            for it in range(MAXITER):
                last = it == MAXITER - 1
                # Self-consistent software pipeline: the first dual of an
                # iteration (plane 2) depends on a tanh from the MIDDLE of
                # the previous iteration (p2 at ~2/3), so iterations overlap
                # by ~9 blocks and no engine drains at the boundary.
                sched = [
                    ("d", 2), ("d", 3), ("ab", 2), ("sb", 2), ("d", 4),
                    ("ab", 3), ("sb", 3), ("p", 3), ("d", 5), ("ab", 4),
                    ("sb", 4), ("p", 4), ("d", 1), ("ab", 5), ("sb", 5),
                    ("p", 5), ("d", 6), ("ab", 1), ("sb", 1), ("p", 2),
                    ("d", 0), ("ab", 6), ("p", 6), ("d", 7), ("ab", 0),
                    ("p", 1), ("ab", 7), ("p", 7), ("p", 0),
                ]
                for kind, i in sched:
                    if kind == "d":
                        if i in (2, 4):
                            emit_mprods(i, pair=True)
                        elif i not in (3, 5):
                            emit_mprods(i)
                        emit_fold(i, it)
                        emit_dual_mm(i, it)
                    elif kind == "ab":
                        emit_ab(i)
                    elif kind == "sb":
                        emit_sb(i)
                    else:
                        emit_primal(i, last)

    nc.compile()
    return nc


def kernel(o, vector_field, nabla_w, div_w):
    global LAST_RESULTS
    if "nc" not in _CACHE:
        _CACHE["nc"] = _build()
    nc = _CACHE["nc"]

    o2 = (2.0 * np.asarray(o, dtype=np.float32)[:, 0]).astype(np.float16)
    vf = np.asarray(vector_field, dtype=np.float32)
    s = np.float32(1.0 / np.sqrt(2.0))
    g0 = (vf[:, :, 0] * s).astype(np.float16)
    g1 = (vf[:, :, 1] * s).astype(np.float16)
    gs = (g0.astype(np.float32) + g1.astype(np.float32)).astype(np.float16)
    # g1s[p] = g1 row (8p-1): partner coefficient for z = g1s * t[plane 0]
    g1s = np.zeros((P, W), dtype=np.float16)
    g1s[1:] = g1[7 : H - NP : NP]
    eye = np.eye(P, dtype=np.float16)
    nsd = np.zeros((P, P), dtype=np.float16)  # out[m] = -in[m+1]
    nsd[np.arange(1, P), np.arange(P - 1)] = -1.0
    spu = np.zeros((P, P), dtype=np.float16)  # out[m] = +in[m-1]
    spu[np.arange(P - 1), np.arange(1, P)] = 1.0
    e127 = np.zeros((P, P), dtype=np.float16)
    e127[P - 1, P - 1] = 1.0
    mats = np.concatenate([eye, -eye, nsd, spu, e127], axis=1)

    in_maps = [
        {"o2": np.ascontiguousarray(o2[b]), "g0": g0, "g1": g1, "gs": gs,
         "g1s": g1s, "mats": mats}
        for b in range(B)
    ]
    res = bass_utils.run_bass_kernel_spmd(nc, in_maps, core_ids=list(range(B)))
    LAST_RESULTS = res
    return np.stack([r["out"] for r in res.results]).astype(np.float32)
